# revision 28
# baseline (speedup 1.0000x reference)
"""Trainium2 Bass kernel for nn_DebugLinear (threshold-pruned linear + stats).

Computes, for x=last_out [8192,4096] f32, W=weight [4096,4096] f32, t=threshold:
    keep = |x| >= t; xp = where(keep, x, 0); c = xp @ W.T
    pruned = x.size - sum(keep); total = x.size

Strategy (8 NeuronCores):
  - tokens (rows of x) sharded 8 ways; W replicated.
  - per core: xT shard [4096,1024] streams in and is pruned on DVE
    (one scalar_tensor_tensor op; abs on the scalar engine), kept
    resident in SBUF as float32r (full-rate fp32 matmul mode).
  - W (host-pretransposed+swizzled) streams through once as the matmul
    stationary operand; PE accumulates K=4096 in PSUM producing c.T
    tiles [128 dout, 512 tok]. The first 4 dout blocks run kt-major,
    interleaved with the prune stream, so the PE stays dense during the
    load phase; zero-valued filler matmuls keep the PE clock gate open.
  - kept-count is recomputed late from xp != 0 (pruned slots are exact
    zeros), off the PE critical path.
  - host re-assembles c from the 8 c.T shards and sums the counts.
"""

import numpy as np

import concourse.bass as bass
import concourse.tile as tile
from concourse import bacc, bass_utils, mybir

N_CORES = 8
N_TOK, D_IN, D_OUT = 8192, 4096, 4096
TOK_SH = N_TOK // N_CORES  # 1024 tokens per core
P = 128
KT = D_IN // P    # 32 k-tiles
DB = D_OUT // P   # 32 dout blocks
STRIP = 512       # moving-operand width (one PSUM bank)
NSTRIP = TOK_SH // STRIP  # 2

_nc_cache = {}


def _build(threshold: float):
    f32 = mybir.dt.float32
    f32r = mybir.dt.float32r
    nc = bacc.Bacc("TRN2", target_bir_lowering=False, debug=False)
    xt_d = nc.dram_tensor("xt", [KT, P, TOK_SH], f32, kind="ExternalInput")
    wsb_d = nc.dram_tensor("wsb", [DB, P, KT * P], f32r, kind="ExternalInput")
    ct_d = nc.dram_tensor("ct", [DB, P, TOK_SH], f32, kind="ExternalOutput")
    cnt_d = nc.dram_tensor("cnt", [P, 1], f32, kind="ExternalOutput")

    with tile.TileContext(nc) as tc:
        QK = 8                    # k-tiles per W chunk
        NQ = KT // QK             # 4 chunks per dout block
        CHUNK = QK * P            # 1024 floats per partition
        G = 4                     # dout blocks co-scheduled with phase A

        def load_chunk(pool, db, q):
            wc = pool.tile([P, CHUNK], f32r, name=f"wc", tag="wc")
            nc.gpsimd.dma_start(wc[:], wsb_d.ap()[db][:, q * CHUNK:(q + 1) * CHUNK])
            return wc

        def evict(ps, db, s, eng="v"):
            ev = ev_p.tile([P, STRIP], f32, name="ev", tag="ev")
            if eng == "v":
                nc.vector.tensor_copy(ev[:], ps[:])
            else:
                nc.scalar.copy(ev[:], ps[:])
            nc.scalar.dma_start(ct_d.ap()[db, :, s * STRIP:(s + 1) * STRIP], ev[:])

        with (
            tc.tile_pool(name="xstage", bufs=8) as xstage_p,
            tc.tile_pool(name="work", bufs=2) as work_p,
            tc.tile_pool(name="xp", bufs=1) as xp_p,
            tc.tile_pool(name="w", bufs=9) as w_p,
            tc.tile_pool(name="cnt", bufs=1) as cnt_p,
            tc.tile_pool(name="warm", bufs=1) as warm_p,
            tc.tile_pool(name="ps", bufs=8, space="PSUM") as ps_p,
            tc.tile_pool(name="ev", bufs=3) as ev_p,
        ):
            cnt32 = cnt_p.tile([P, 2 * KT], f32, name="cnt32", tag="cnt32")
            xp_tiles = []

            # PE warm-up: ~4us of dense dummy matmuls at t=0 releases the
            # HAM clock gate (cold PE runs at half clock) before real work
            # arrives. Data values are irrelevant; the psum is never read.
            wu_w = warm_p.tile([P, P], f32r, name="wu_w", tag="wu_w")
            wu_x = warm_p.tile([P, P], f32r, name="wu_x", tag="wu_x")
            wu_s = work_p.tile([P, P], f32, name="wu_s", tag="scratch")
            nc.vector.memset(wu_s[:], 0.0)
            nc.vector.scalar_tensor_tensor(
                wu_x[:], wu_s[:], 1.0, wu_s[:],
                mybir.AluOpType.is_ge, mybir.AluOpType.mult,
            )
            nc.vector.scalar_tensor_tensor(
                wu_w[:], wu_s[:], 1.0, wu_s[:],
                mybir.AluOpType.is_ge, mybir.AluOpType.mult,
            )
            wu_ps = ps_p.tile([P, STRIP], f32, name="wu_ps", tag="ps")
            for _ in range(40):
                nc.tensor.matmul(wu_ps[:, 0:P], wu_w[:], wu_x[:], start=True, stop=True)
            # Phase A (load + prune), interleaved kt-major with the first G
            # dout blocks so the PE stays dense while activations stream in.
            # x streams in strip-wide half tiles (deep DMA pipelining);
            # xp = (|x| >= t) * x in one DVE op; abs on the scalar engine.
            g0_ps = [
                ps_p.tile([P, STRIP], f32, name=f"ps0_{db}_{s}", tag="ps")
                for db in range(G) for s in range(NSTRIP)
            ]
            g0_w = {}
            for kt in range(KT):
                xp = xp_p.tile([P, TOK_SH], f32r, name=f"xp{kt}", tag=f"xp{kt}")
                xp_tiles.append(xp)
                if kt % QK == 0:
                    q = kt // QK
                    for db in range(G):
                        g0_w[db] = load_chunk(w_p, db, q)
                    if q == NQ - 1:
                        # prefetch the first steady-state dout block's W
                        g1_chunks = [load_chunk(w_p, G, qq) for qq in range(NQ)]
                for s in range(NSTRIP):
                    xraw = xstage_p.tile([P, STRIP], f32, name="xraw", tag="xraw")
                    nc.sync.dma_start(xraw[:], xt_d.ap()[kt][:, s * STRIP:(s + 1) * STRIP])
                    absx = work_p.tile([P, STRIP], f32, name="absx", tag="scratch")
                    nc.scalar.activation(absx[:], xraw[:], mybir.ActivationFunctionType.Abs)
                    nc.vector.scalar_tensor_tensor(
                        xp[:, s * STRIP:(s + 1) * STRIP], absx[:], threshold, xraw[:],
                        mybir.AluOpType.is_ge, mybir.AluOpType.mult,
                    )
                    for db in range(G):
                        nc.tensor.matmul(
                            g0_ps[db * NSTRIP + s][:],
                            g0_w[db][:, (kt % QK) * P:(kt % QK + 1) * P],
                            xp[:, s * STRIP:(s + 1) * STRIP],
                            start=(kt == 0), stop=(kt == KT - 1),
                        )
                    # Zero-valued filler matmuls (+0 into a live psum group)
                    # keep the PE activity monitor from dropping to half
                    # clock while supply-gated. Numerically a no-op.
                    if 0 < kt < KT - 1:
                        nfill = 5 if kt < 4 else 3
                        for f in range(nfill):
                            nc.tensor.matmul(
                                g0_ps[(kt + f) % (G * NSTRIP)][:, 0:P],
                                wu_w[:], wu_x[:], start=False, stop=False,
                            )
            for db in range(G):
                for s in range(NSTRIP):
                    evict(g0_ps[db * NSTRIP + s], db, s, eng="s" if s == 0 else "v")

            # Steady state: remaining dout blocks, db-major. W loads for
            # db > G are gated past the load phase so they don't steal HBM
            # bandwidth from the activation stream (the slot-limited
            # prefetch pipeline still runs well ahead of consumption).
            for db in range(G, DB):
                if db == G:
                    chunks = g1_chunks
                else:
                    with tc.tile_wait_until(0.09):
                        chunks = [load_chunk(w_p, db, q) for q in range(NQ)]
                for s in range(NSTRIP):
                    ps = ps_p.tile([P, STRIP], f32, name="ps", tag="ps")
                    for kt in range(KT):
                        nc.tensor.matmul(
                            ps[:],
                            chunks[kt // QK][:, (kt % QK) * P:(kt % QK + 1) * P],
                            xp_tiles[kt][:, s * STRIP:(s + 1) * STRIP],
                            start=(kt == 0), stop=(kt == KT - 1),
                        )
                    evict(ps, db, s)

            # Deferred stats (off the PE critical path): pruned slots of xp
            # are exactly +/-0, kept slots are nonzero, so kept = sum(xp != 0).
            # Gate the count ops behind a virtual timestamp so the scheduler
            # cannot slot them onto the Vector engine ahead of the group-0
            # psum evictions (which would stall PSUM recycling).
            with tc.tile_wait_until(0.15):
                for kt in range(KT):
                    for s in range(NSTRIP):
                        mcnt = work_p.tile([P, STRIP], f32, name="mcnt", tag="scratch")
                        nc.vector.tensor_scalar(
                            mcnt[:], xp_tiles[kt][:, s * STRIP:(s + 1) * STRIP].bitcast(f32), 0.0, 0.0,
                            mybir.AluOpType.not_equal, mybir.AluOpType.add,
                            accum_out=cnt32[:, 2 * kt + s:2 * kt + s + 1],
                        )
            cnt1 = cnt_p.tile([P, 1], f32, name="cnt1", tag="cnt1")
            nc.vector.tensor_reduce(cnt1[:], cnt32[:], mybir.AxisListType.X, mybir.AluOpType.add)
            nc.sync.dma_start(cnt_d.ap(), cnt1[:])
    nc.compile()
    return nc


def _get_nc(threshold: float):
    key = float(threshold)
    if key not in _nc_cache:
        _nc_cache[key] = _build(key)
    return _nc_cache[key]


def kernel(last_out, weight, threshold, _want_trace=False):
    last_out = np.ascontiguousarray(np.asarray(last_out, dtype=np.float32))
    weight = np.ascontiguousarray(np.asarray(weight, dtype=np.float32))
    thr = float(np.asarray(threshold).reshape(-1)[0])

    nc = _get_nc(thr)

    # W swizzle: wsb[db, p, kt*128+m] = weight[db*128+m, kt*128+p]
    wsb = np.ascontiguousarray(
        weight.reshape(DB, P, KT, P).transpose(0, 3, 2, 1).reshape(DB, P, KT * P)
    )

    in_maps = []
    for i in range(N_CORES):
        xs = last_out[i * TOK_SH:(i + 1) * TOK_SH, :]       # [1024, 4096]
        xt = np.ascontiguousarray(xs.T).reshape(KT, P, TOK_SH)
        in_maps.append({"xt": xt, "wsb": wsb})

    res = None
    for attempt in range(3):
        try:
            res = bass_utils.run_bass_kernel_spmd(
                nc, in_maps, core_ids=list(range(N_CORES)), trace=_want_trace,
            )
            break
        except Exception:
            # Transient NRT device errors (e.g. a wedged core from a prior
            # aborted run) usually clear on retry.
            if attempt == 2:
                raise

    c = np.empty((N_TOK, D_OUT), dtype=np.float32)
    kept = 0.0
    for i in range(N_CORES):
        ct = res.results[i]["ct"].reshape(D_OUT, TOK_SH)
        c[i * TOK_SH:(i + 1) * TOK_SH, :] = ct.T
        kept += float(res.results[i]["cnt"].astype(np.float64).sum())

    total = np.int32(N_TOK * D_IN)
    pruned = np.int32(int(N_TOK * D_IN - kept))
    if _want_trace:
        kernel.last_results = res
    return c, pruned, total


# revision 29
# speedup vs baseline: 1.0031x; 1.0031x over previous
"""Trainium2 Bass kernel for nn_DebugLinear (threshold-pruned linear + stats).

Computes, for x=last_out [8192,4096] f32, W=weight [4096,4096] f32, t=threshold:
    keep = |x| >= t; xp = where(keep, x, 0); c = xp @ W.T
    pruned = x.size - sum(keep); total = x.size

Strategy (8 NeuronCores):
  - tokens (rows of x) sharded 8 ways; W replicated.
  - per core: xT shard [4096,1024] streams in and is pruned on DVE
    (one scalar_tensor_tensor op; abs on the scalar engine), kept
    resident in SBUF as float32r (full-rate fp32 matmul mode).
  - W (host-pretransposed+swizzled) streams through once as the matmul
    stationary operand; PE accumulates K=4096 in PSUM producing c.T
    tiles [128 dout, 512 tok]. The first 4 dout blocks run kt-major,
    interleaved with the prune stream, so the PE stays dense during the
    load phase; zero-valued filler matmuls keep the PE clock gate open.
  - kept-count is recomputed late from xp != 0 (pruned slots are exact
    zeros), off the PE critical path.
  - host re-assembles c from the 8 c.T shards and sums the counts.
"""

import numpy as np

import concourse.bass as bass
import concourse.tile as tile
from concourse import bacc, bass_utils, mybir

N_CORES = 8
N_TOK, D_IN, D_OUT = 8192, 4096, 4096
TOK_SH = N_TOK // N_CORES  # 1024 tokens per core
P = 128
KT = D_IN // P    # 32 k-tiles
DB = D_OUT // P   # 32 dout blocks
STRIP = 512       # moving-operand width (one PSUM bank)
NSTRIP = TOK_SH // STRIP  # 2

_nc_cache = {}


def _build(threshold: float):
    f32 = mybir.dt.float32
    f32r = mybir.dt.float32r
    nc = bacc.Bacc("TRN2", target_bir_lowering=False, debug=False)
    xt_d = nc.dram_tensor("xt", [KT, P, TOK_SH], f32, kind="ExternalInput")
    wsb_d = nc.dram_tensor("wsb", [DB, P, KT * P], f32r, kind="ExternalInput")
    ct_d = nc.dram_tensor("ct", [DB, P, TOK_SH], f32, kind="ExternalOutput")
    cnt_d = nc.dram_tensor("cnt", [P, 1], f32, kind="ExternalOutput")

    with tile.TileContext(nc) as tc:
        QK = 8                    # k-tiles per W chunk
        NQ = KT // QK             # 4 chunks per dout block
        CHUNK = QK * P            # 1024 floats per partition
        G = 4                     # dout blocks co-scheduled with phase A

        def load_chunk(pool, db, q):
            wc = pool.tile([P, CHUNK], f32r, name=f"wc", tag="wc")
            nc.gpsimd.dma_start(wc[:], wsb_d.ap()[db][:, q * CHUNK:(q + 1) * CHUNK])
            return wc

        def evict(ps, db, s, eng="v"):
            ev = ev_p.tile([P, STRIP], f32, name="ev", tag="ev")
            if eng == "v":
                nc.vector.tensor_copy(ev[:], ps[:])
            else:
                nc.scalar.copy(ev[:], ps[:])
            nc.scalar.dma_start(ct_d.ap()[db, :, s * STRIP:(s + 1) * STRIP], ev[:])

        with (
            tc.tile_pool(name="xstage", bufs=8) as xstage_p,
            tc.tile_pool(name="work", bufs=2) as work_p,
            tc.tile_pool(name="xp", bufs=1) as xp_p,
            tc.tile_pool(name="w", bufs=9) as w_p,
            tc.tile_pool(name="cnt", bufs=1) as cnt_p,
            tc.tile_pool(name="warm", bufs=1) as warm_p,
            tc.tile_pool(name="ps", bufs=8, space="PSUM") as ps_p,
            tc.tile_pool(name="ev", bufs=3) as ev_p,
        ):
            cnt32 = cnt_p.tile([P, 2 * KT], f32, name="cnt32", tag="cnt32")
            xp_tiles = []

            # PE warm-up: ~4us of dense dummy matmuls at t=0 releases the
            # HAM clock gate (cold PE runs at half clock) before real work
            # arrives. Data values are irrelevant; the psum is never read.
            wu_w = warm_p.tile([P, P], f32r, name="wu_w", tag="wu_w")
            wu_x = warm_p.tile([P, P], f32r, name="wu_x", tag="wu_x")
            wu_s = work_p.tile([P, P], f32, name="wu_s", tag="scratch")
            nc.vector.memset(wu_s[:], 0.0)
            nc.vector.scalar_tensor_tensor(
                wu_x[:], wu_s[:], 1.0, wu_s[:],
                mybir.AluOpType.is_ge, mybir.AluOpType.mult,
            )
            nc.vector.scalar_tensor_tensor(
                wu_w[:], wu_s[:], 1.0, wu_s[:],
                mybir.AluOpType.is_ge, mybir.AluOpType.mult,
            )
            wu_ps = ps_p.tile([P, STRIP], f32, name="wu_ps", tag="ps")
            for _ in range(24):
                nc.tensor.matmul(wu_ps[:, 0:P], wu_w[:], wu_x[:], start=True, stop=True)
            # Phase A (load + prune), interleaved kt-major with the first G
            # dout blocks so the PE stays dense while activations stream in.
            # x streams in strip-wide half tiles (deep DMA pipelining);
            # xp = (|x| >= t) * x in one DVE op; abs on the scalar engine.
            g0_ps = [
                ps_p.tile([P, STRIP], f32, name=f"ps0_{db}_{s}", tag="ps")
                for db in range(G) for s in range(NSTRIP)
            ]
            g0_w = {}
            for kt in range(KT):
                xp = xp_p.tile([P, TOK_SH], f32r, name=f"xp{kt}", tag=f"xp{kt}")
                xp_tiles.append(xp)
                if kt % QK == 0:
                    q = kt // QK
                    for db in range(G):
                        g0_w[db] = load_chunk(w_p, db, q)
                    if q == NQ - 1:
                        # prefetch the first steady-state dout block's W
                        g1_chunks = [load_chunk(w_p, G, qq) for qq in range(NQ)]
                for s in range(NSTRIP):
                    xraw = xstage_p.tile([P, STRIP], f32, name="xraw", tag="xraw")
                    nc.sync.dma_start(xraw[:], xt_d.ap()[kt][:, s * STRIP:(s + 1) * STRIP])
                    absx = work_p.tile([P, STRIP], f32, name="absx", tag="scratch")
                    nc.scalar.activation(absx[:], xraw[:], mybir.ActivationFunctionType.Abs)
                    nc.vector.scalar_tensor_tensor(
                        xp[:, s * STRIP:(s + 1) * STRIP], absx[:], threshold, xraw[:],
                        mybir.AluOpType.is_ge, mybir.AluOpType.mult,
                    )
                    for db in range(G):
                        nc.tensor.matmul(
                            g0_ps[db * NSTRIP + s][:],
                            g0_w[db][:, (kt % QK) * P:(kt % QK + 1) * P],
                            xp[:, s * STRIP:(s + 1) * STRIP],
                            start=(kt == 0), stop=(kt == KT - 1),
                        )
                    # Zero-valued filler matmuls (+0 into a live psum group)
                    # keep the PE activity monitor from dropping to half
                    # clock while supply-gated. Numerically a no-op.
                    if 0 < kt < KT - 1:
                        nfill = 2 if kt < 8 else 1
                        for f in range(nfill):
                            nc.tensor.matmul(
                                g0_ps[(kt + f) % (G * NSTRIP)][:, 0:P],
                                wu_w[:], wu_x[:], start=False, stop=False,
                            )
            for db in range(G):
                for s in range(NSTRIP):
                    evict(g0_ps[db * NSTRIP + s], db, s, eng="s" if s == 0 else "v")

            # Steady state: remaining dout blocks, db-major. W loads for
            # db > G are gated past the load phase so they don't steal HBM
            # bandwidth from the activation stream (the slot-limited
            # prefetch pipeline still runs well ahead of consumption).
            for db in range(G, DB):
                if db == G:
                    chunks = g1_chunks
                else:
                    with tc.tile_wait_until(0.09):
                        chunks = [load_chunk(w_p, db, q) for q in range(NQ)]
                for s in range(NSTRIP):
                    ps = ps_p.tile([P, STRIP], f32, name="ps", tag="ps")
                    for kt in range(KT):
                        nc.tensor.matmul(
                            ps[:],
                            chunks[kt // QK][:, (kt % QK) * P:(kt % QK + 1) * P],
                            xp_tiles[kt][:, s * STRIP:(s + 1) * STRIP],
                            start=(kt == 0), stop=(kt == KT - 1),
                        )
                    evict(ps, db, s)

            # Deferred stats (off the PE critical path): pruned slots of xp
            # are exactly +/-0, kept slots are nonzero, so kept = sum(xp != 0).
            # Gate the count ops behind a virtual timestamp so the scheduler
            # cannot slot them onto the Vector engine ahead of the group-0
            # psum evictions (which would stall PSUM recycling).
            with tc.tile_wait_until(0.15):
                for kt in range(KT):
                    for s in range(NSTRIP):
                        mcnt = work_p.tile([P, STRIP], f32, name="mcnt", tag="scratch")
                        nc.vector.tensor_scalar(
                            mcnt[:], xp_tiles[kt][:, s * STRIP:(s + 1) * STRIP].bitcast(f32), 0.0, 0.0,
                            mybir.AluOpType.not_equal, mybir.AluOpType.add,
                            accum_out=cnt32[:, 2 * kt + s:2 * kt + s + 1],
                        )
            cnt1 = cnt_p.tile([P, 1], f32, name="cnt1", tag="cnt1")
            nc.vector.tensor_reduce(cnt1[:], cnt32[:], mybir.AxisListType.X, mybir.AluOpType.add)
            nc.sync.dma_start(cnt_d.ap(), cnt1[:])
    nc.compile()
    return nc


def _get_nc(threshold: float):
    key = float(threshold)
    if key not in _nc_cache:
        _nc_cache[key] = _build(key)
    return _nc_cache[key]


def kernel(last_out, weight, threshold, _want_trace=False):
    last_out = np.ascontiguousarray(np.asarray(last_out, dtype=np.float32))
    weight = np.ascontiguousarray(np.asarray(weight, dtype=np.float32))
    thr = float(np.asarray(threshold).reshape(-1)[0])

    nc = _get_nc(thr)

    # W swizzle: wsb[db, p, kt*128+m] = weight[db*128+m, kt*128+p]
    wsb = np.ascontiguousarray(
        weight.reshape(DB, P, KT, P).transpose(0, 3, 2, 1).reshape(DB, P, KT * P)
    )

    in_maps = []
    for i in range(N_CORES):
        xs = last_out[i * TOK_SH:(i + 1) * TOK_SH, :]       # [1024, 4096]
        xt = np.ascontiguousarray(xs.T).reshape(KT, P, TOK_SH)
        in_maps.append({"xt": xt, "wsb": wsb})

    res = None
    for attempt in range(3):
        try:
            res = bass_utils.run_bass_kernel_spmd(
                nc, in_maps, core_ids=list(range(N_CORES)), trace=_want_trace,
            )
            break
        except Exception:
            # Transient NRT device errors (e.g. a wedged core from a prior
            # aborted run) usually clear on retry.
            if attempt == 2:
                raise

    c = np.empty((N_TOK, D_OUT), dtype=np.float32)
    kept = 0.0
    for i in range(N_CORES):
        ct = res.results[i]["ct"].reshape(D_OUT, TOK_SH)
        c[i * TOK_SH:(i + 1) * TOK_SH, :] = ct.T
        kept += float(res.results[i]["cnt"].astype(np.float64).sum())

    total = np.int32(N_TOK * D_IN)
    pruned = np.int32(int(N_TOK * D_IN - kept))
    if _want_trace:
        kernel.last_results = res
    return c, pruned, total
